# revision 21
# baseline (speedup 1.0000x reference)
"""ViT attention block (B=8, N=1024, dim=1024, heads=16, d_k=64) on 8 trn2 NeuronCores.

Sharding: data-parallel over batch (1 batch per core), weights replicated.
No collectives; each core computes its batch's full attention output.

Schedule (vs the original per-tile-streaming version): all weights are
SBUF-resident via consolidated startup DMAs on the sync/HWDGE queue;
the S^T PSUM is split per head and double-buffered so exp(h0) overlaps
S^T(h1); fillers use a single-bank per-nh PSUM in consumption-priority
order (KT-nh0 first); normalize DMAs ride the gpsimd SWDGE queue (sync
for the last two pairs) with the reciprocal on compact [128,8] data;
V j6/j7 matmuls fill pair-0 slots; PV(6) is front-loaded into pair-7
slots 0-3 and PV(7) chunks dribble through slots 4-7 as their exps
land; the projection interleaves with the PV(7) drain with four j-tile
PSUMs in flight (pst ring + ppv halves + fill bank).

Per-core algorithm (all matmuls contract over the partition dim):
  - host pre-transposes x[b] -> xT [dim, tokens].
  - QT/KT = (w_qkv[:, :2048]).T @ xT -> [2048, tokens]; head pair 2t,2t+1
    lives in partition-tile t ([128, 1024]).
  - V = xT.T @ w_qkv[:, 2048:] -> [tokens, 1024], stored with a ones column
    per head (65 cols/head) so PV produces softmax row-sums for free.
  - S^T = KT.T @ QT per head (row-group pairs via tile_position run
    concurrently on the PE); exp on ScalarE straight out of PSUM -> E bf16.
    (max-subtraction skipped: |scale*S| <~ 2, exp is exact-safe and softmax
    is shift-invariant.)
  - PV: out^T[d'+1, n] = V'.T @ E accumulated over m tiles; row 64 is the
    denominator. Staged to SBUF bf16; denominator row is broadcast across
    64 partitions with a K=1 ones matmul into PSUM, reciprocal'd on DVE,
    and fused into the normalize multiply. Odd head's rows reach partitions
    64-127 of attnT via a local SBUF->SBUF DMA.
  - final = attnT.T @ w_out + b_out, fp32 out.
"""

import os
import numpy as np
import ml_dtypes

import concourse.bass as bass
from concourse import bacc
import concourse.mybir as mybir
import concourse.tile as tile
from concourse.bass_utils import run_bass_kernel_spmd

P = 128
N_TOK = 1024
DIM = 1024
HEADS = 16
D_K = 64
N_CORES = 8
SCALE = D_K ** -0.5  # 0.125

NP_T = N_TOK // P   # 8 token tiles
DP = DIM // P       # 8 dim tiles
NPAIRS = HEADS // 2  # 8 head pairs
VW = D_K + 1        # 65: V columns per head incl. ones column

MM_DTYPE = os.environ.get("KERNEL_MM_DTYPE", "bf16")
_DT = {
    "bf16": mybir.dt.bfloat16,
    "fp32r": mybir.dt.float32r,
    "fp32": mybir.dt.float32,
}[MM_DTYPE]
_NPDT = {"bf16": ml_dtypes.bfloat16, "fp32r": np.float32, "fp32": np.float32}[MM_DTYPE]

F32 = mybir.dt.float32


def build_program():
    nc = bacc.Bacc("TRN2", target_bir_lowering=False, debug=False)

    xT = nc.dram_tensor("xT", [DIM, N_TOK], _DT, kind="ExternalInput").ap()
    wqkv = nc.dram_tensor("w_qkv", [DIM, 3 * DIM], _DT, kind="ExternalInput").ap()
    wout = nc.dram_tensor("w_out", [DIM, DIM], _DT, kind="ExternalInput").ap()
    bout = nc.dram_tensor("b_out", [DIM], F32, kind="ExternalInput").ap()
    out = nc.dram_tensor("out", [N_TOK, DIM], F32, kind="ExternalOutput").ap()
    # denominator bounce buffers (raw row, then reciprocal row)
    rs_dram = nc.dram_tensor("rs_scratch", [HEADS, N_TOK], _DT).ap()
    rs2_dram = nc.dram_tensor("rs2_scratch", [HEADS, N_TOK], F32).ap()

    with tile.TileContext(nc) as tc:
        with (
            tc.tile_pool(name="persist", bufs=1) as persist,
            tc.tile_pool(name="qkt", bufs=5) as qktp,
            tc.tile_pool(name="et", bufs=20) as etp,
            tc.tile_pool(name="stg", bufs=3) as stgp,
            tc.tile_pool(name="small", bufs=2) as small,
            tc.tile_pool(name="ev", bufs=2) as evp,
        ):
            # ---------------- persistent SBUF tensors ----------------
            xT_sb = persist.tile([P, DP * N_TOK], _DT, tag="xT")     # cols k*1024+n
            wqk_sb = persist.tile([P, 16 * N_TOK], _DT, tag="wqk")   # cols j*1024+k*128+c
            wv_sb = persist.tile([P, DP * DIM], _DT, tag="wv")       # cols k*1024+c
            wout_sb = persist.tile([P, DP * DIM], _DT, tag="wout")   # cols k*1024+c
            bias_bc = persist.tile([P, DIM], F32, tag="bias")
            v_sb = []
            attnT_sb = []
            for j in range(NP_T):
                v_sb.append(persist.tile([P, HEADS * VW], _DT, tag=f"v{j}",
                                         name=f"v{j}"))
            for p in range(NPAIRS):
                attnT_sb.append(persist.tile([P, N_TOK], _DT, tag=f"attnT{p}",
                                             name=f"attnT{p}"))

            def xs(k, lo, n):
                return xT_sb[:, k * N_TOK + lo:k * N_TOK + lo + n]

            def wqks(j, k):
                return wqk_sb[:, j * N_TOK + k * P:j * N_TOK + (k + 1) * P]

            # ---------------- startup DMAs ----------------
            # sync queue: xT halves, QK weights per j (pair-ordered), wout.
            # gpsimd queue: V weights, bias.
            def dma_xT(k0, nk):
                src = bass.AP(tensor=xT.tensor, offset=k0 * P * N_TOK,
                              ap=[[N_TOK, P], [P * N_TOK, nk], [1, N_TOK]])
                nc.sync.dma_start(xT_sb[:, k0 * N_TOK:(k0 + nk) * N_TOK], src)

            def dma_wqk(j):
                # j in 0..15; source cols j*128..j*128+127 over all 8 k blocks
                src = bass.AP(tensor=wqkv.tensor, offset=j * P,
                              ap=[[3 * DIM, P], [P * 3 * DIM, DP], [1, P]])
                nc.sync.dma_start(wqk_sb[:, j * N_TOK:(j + 1) * N_TOK], src)

            def dma_wv(k0, nk):
                src = bass.AP(tensor=wqkv.tensor, offset=2 * DIM + k0 * P * 3 * DIM,
                              ap=[[3 * DIM, P], [P * 3 * DIM, nk], [1, DIM]])
                nc.sync.dma_start(wv_sb[:, k0 * DIM:(k0 + nk) * DIM], src)

            def dma_wout(k0, nk):
                src = bass.AP(tensor=wout.tensor, offset=k0 * P * DIM,
                              ap=[[DIM, P], [P * DIM, nk], [1, DIM]])
                nc.sync.dma_start(wout_sb[:, k0 * DIM:(k0 + nk) * DIM], src)

            dma_xT(0, 1)
            dma_wv(0, 1)
            dma_xT(1, 1)
            dma_wv(1, 1)
            dma_xT(2, 2)
            dma_wv(2, 2)
            dma_xT(4, 2)
            dma_wv(4, 2)
            dma_xT(6, 2)
            dma_wv(6, 2)
            dma_wqk(0)
            dma_wqk(8)
            for pn in range(1, NPAIRS):
                dma_wqk(pn)
                dma_wqk(8 + pn)
            dma_wout(0, 4)
            dma_wout(4, 4)
            bias_in = bass.AP(tensor=bout.tensor, offset=bout.offset,
                              ap=[[0, P]] + list(bout.ap))
            nc.gpsimd.dma_start(bias_bc[:], bias_in)
            for j in range(NP_T):
                nc.vector.memset(
                    v_sb[j][:].rearrange("p (h x) -> p h x", x=VW)[:, :, D_K:],
                    1.0)

            # ============ phase 1: V j0..j5 and pair-0 QT/KT ============
            with tc.tile_pool(name="pq1", bufs=2, space="PSUM") as pq1:
                for j in range(6):
                    ps = pq1.tile([P, DIM], F32, tag="pq", name=f"psv{j}")
                    for k in range(DP):
                        for nh in range(2):
                            nc.tensor.matmul(
                                ps[:, nh * 512:(nh + 1) * 512],
                                lhsT=xs(k, j * P, P),
                                rhs=wv_sb[:, k * DIM + nh * 512:
                                          k * DIM + (nh + 1) * 512],
                                start=(k == 0), stop=(k == DP - 1),
                            )
                    nc.vector.tensor_copy(
                        out=v_sb[j][:].rearrange("p (h x) -> p h x", x=VW)[:, :, :D_K],
                        in_=ps[:].rearrange("p (h d) -> p h d", d=D_K),
                    )
                qkt_tiles = {}
                for j in (0, 8):
                    ps = pq1.tile([P, N_TOK], F32, tag="pq", name=f"psqk{j}")
                    for k in range(DP):
                        for nh in range(2):
                            nc.tensor.matmul(
                                ps[:, nh * 512:(nh + 1) * 512],
                                lhsT=wqks(j, k),
                                rhs=xs(k, nh * 512, 512),
                                start=(k == 0), stop=(k == DP - 1),
                            )
                    t = qktp.tile([P, N_TOK], _DT, tag="qkt", name=f"qkt{j}")
                    nc.vector.tensor_copy(out=t[:], in_=ps[:])
                    qkt_tiles[j] = t
                qt_cur, kt_cur = qkt_tiles[0], qkt_tiles[8]

            # ============ phase 2: pipelined attention ============
            with (
                tc.tile_pool(name="pst", bufs=2, space="PSUM") as pst,
                tc.tile_pool(name="fillp", bufs=1, space="PSUM") as fillp,
                tc.tile_pool(name="ppv", bufs=3, space="PSUM") as ppv,
            ):
                et_tiles = {}   # (pair, mt, h) -> [128, 1024] bf16
                inflight = {}

                def normalize(p, h, stg):
                    """Denominator halves were DMA'd to rs_dram straight from
                    the PV PSUMs; reshape-load to [128, 8] for a compact
                    reciprocal, bounce back, and broadcast-read into the
                    normalize multiply. Late pairs ride the (then idle) sync
                    queue; earlier ones use gpsimd SWDGE."""
                    hg = 2 * p + h
                    dma = nc.sync.dma_start if p >= NPAIRS - 2 else \
                        nc.gpsimd.dma_start
                    rsp = small.tile([P, NP_T], _DT, tag="rsp", name=f"rsp{hg}")
                    dma(rsp[:], rs_dram[hg].rearrange("(p i) -> p i", p=P))
                    rspr = small.tile([P, NP_T], F32, tag="rspr",
                                      name=f"rspr{hg}")
                    nc.vector.reciprocal(rspr[:], rsp[:])
                    dma(rs2_dram[hg].rearrange("(p i) -> p i", p=P), rspr[:])
                    rs_row = rs2_dram[hg:hg + 1, :]
                    rs_bc = bass.AP(tensor=rs_row.tensor, offset=rs_row.offset,
                                    ap=[[0, D_K], list(rs_row.ap)[-1]])
                    rcp = small.tile([D_K, N_TOK], F32, tag="rcp",
                                     name=f"rcp{hg}")
                    dma(rcp[:], rs_bc)
                    if h == 0:
                        nc.vector.tensor_mul(out=attnT_sb[p][0:D_K, :],
                                             in0=stg[0:D_K, :], in1=rcp[:])
                    else:
                        tmp = small.tile([D_K, N_TOK], _DT, tag="oddtmp",
                                         name=f"oddtmp{hg}")
                        nc.vector.tensor_mul(out=tmp[:],
                                             in0=stg[0:D_K, :], in1=rcp[:])
                        dma(attnT_sb[p][D_K:P, :], tmp[:])

                def norm_half(p, h, stg, nh, tmp):
                    """Pair-7 fast path: per-half reciprocal chain, the two
                    halves on different DMA queues so their issue costs
                    overlap (nh0: gpsimd, nh1: sync)."""
                    hg = 2 * p + h
                    dma = nc.gpsimd.dma_start if nh == 0 else nc.sync.dma_start
                    half_off = hg * N_TOK + nh * 512
                    rsp = small.tile([P, NP_T // 2], _DT, tag="rsp",
                                     name=f"rsph{hg}_{nh}")
                    dma(rsp[:], bass.AP(tensor=rs_dram.tensor,
                                        offset=rs_dram.offset + half_off,
                                        ap=[[NP_T // 2, P], [1, NP_T // 2]]))
                    rspr = small.tile([P, NP_T // 2], F32, tag="rspr",
                                      name=f"rsprh{hg}_{nh}")
                    nc.vector.reciprocal(rspr[:], rsp[:])
                    dma(bass.AP(tensor=rs2_dram.tensor,
                                offset=rs2_dram.offset + half_off,
                                ap=[[NP_T // 2, P], [1, NP_T // 2]]),
                        rspr[:])
                    rcp = small.tile([D_K, 512], F32, tag="rcp",
                                     name=f"rcph{hg}_{nh}")
                    dma(rcp[:], bass.AP(tensor=rs2_dram.tensor,
                                        offset=rs2_dram.offset + half_off,
                                        ap=[[0, D_K], [1, 512]]))
                    sl = slice(nh * 512, (nh + 1) * 512)
                    if h == 0:
                        nc.vector.tensor_mul(out=attnT_sb[p][0:D_K, sl],
                                             in0=stg[0:D_K, sl], in1=rcp[:])
                    else:
                        nc.vector.tensor_mul(out=tmp[:, sl],
                                             in0=stg[0:D_K, sl], in1=rcp[:])
                        dma(attnT_sb[p][D_K:P, sl], tmp[:, sl])

                def pv_chunk(p, slot8, pool=None, mts=None):
                    """4 PV matmuls for pair p: (h, nh) = slot8//4, (slot8//2)%2,
                    half = slot8%2 selects m-tiles 0-3 / 4-7 (or explicit mts)."""
                    h, nh = slot8 // 4, (slot8 // 2) % 2
                    hg = 2 * p + h
                    half = slot8 % 2
                    if mts is None:
                        mts = range(4 * half, 4 * half + 4)
                    key = (p, h, nh)
                    if key not in inflight:
                        inflight[key] = (pool or ppv).tile(
                            [VW, 512], F32, tag="ppv" if pool is None else "fill",
                            name=f"pv{p}_{h}_{nh}")
                    pvt = inflight[key]
                    done = False
                    for mt in mts:
                        nc.tensor.matmul(
                            pvt[:],
                            lhsT=v_sb[mt][:, hg * VW:(hg + 1) * VW],
                            rhs=et_tiles[(p, mt, h)][:, nh * 512:(nh + 1) * 512],
                            start=(mt == 0), stop=(mt == NP_T - 1),
                        )
                        if mt == NP_T - 1:
                            done = True
                    if done:
                        dma = nc.sync.dma_start if p >= NPAIRS - 2 else \
                            nc.gpsimd.dma_start
                        skey = ("stg", p, h)
                        if skey not in inflight:
                            inflight[skey] = stgp.tile(
                                [VW, N_TOK], _DT, tag="stg", name=f"stg{hg}")
                        stg = inflight[skey]
                        nc.vector.tensor_copy(
                            out=stg[:, nh * 512:(nh + 1) * 512], in_=pvt[:])
                        dma(rs_dram[hg:hg + 1, nh * 512:(nh + 1) * 512],
                            stg[D_K:VW, nh * 512:(nh + 1) * 512])
                        del inflight[key]
                        if p == NPAIRS - 1:
                            tkey = ("tmp", p, h)
                            if h == 1 and tkey not in inflight:
                                inflight[tkey] = small.tile(
                                    [D_K, N_TOK], _DT, tag="oddtmp",
                                    name=f"oddtmp{hg}")
                            norm_half(p, h, stg, nh, inflight.get(tkey))
                            if nh == 1:
                                del inflight[skey]
                                if h == 1:
                                    del inflight[tkey]
                        elif nh == 1:
                            normalize(p, h, stg)
                            del inflight[skey]

                def st_exp(p, mt):
                    """S^T + exp for (p, mt): per-head PSUM tiles (ring of 2)
                    so exp(h0) overlaps S^T(h1); heads emitted interleaved so
                    the row-group matmuls pair up on the PE."""
                    sts = []
                    ets = []
                    for h in range(2):
                        sts.append(pst.tile([P, N_TOK], F32, tag="pst",
                                            name=f"st{p}_{mt}_{h}"))
                        e = etp.tile([P, N_TOK], _DT, tag="et",
                                     name=f"et{p}_{mt}_{h}")
                        ets.append(e)
                        et_tiles[(p, mt, h)] = e
                    for nh in range(2):
                        for h in range(2):
                            nc.tensor.matmul(
                                sts[h][:, nh * 512:(nh + 1) * 512],
                                lhsT=kt_cur[h * D_K:(h + 1) * D_K,
                                            mt * P:(mt + 1) * P],
                                rhs=qt_cur[h * D_K:(h + 1) * D_K,
                                           nh * 512:(nh + 1) * 512],
                                start=True, stop=True,
                                tile_position=(h * D_K, 0),
                            )
                    for h in range(2):
                        nc.scalar.activation(ets[h][:], sts[h][:],
                                             mybir.ActivationFunctionType.Exp,
                                             scale=float(SCALE))

                # filler groups: (j, nh) in consumption-priority order:
                # KT-nh0 feeds S^T mt0-3 lhsT, QT halves feed rhs, KT-nh1 last.
                def filler(pnext, mt):
                    g, step = mt // 2, mt % 2
                    is_kt, nh = [(True, 0), (False, 0), (False, 1), (True, 1)][g]
                    j = (NP_T + pnext) if is_kt else pnext
                    key = ("fill", pnext, g)
                    if step == 0:
                        inflight[key] = fillp.tile([P, 512], F32, tag="fill",
                                                   name=f"psf{j}_{nh}")
                    ps = inflight[key]
                    for k in range(4 * step, 4 * step + 4):
                        nc.tensor.matmul(
                            ps[:],
                            lhsT=wqks(j, k),
                            rhs=xs(k, nh * 512, 512),
                            start=(k == 0), stop=(k == DP - 1),
                        )
                    if step == 1:
                        tkey = ("qkt", pnext, is_kt)
                        if tkey not in inflight:
                            inflight[tkey] = qktp.tile([P, N_TOK], _DT,
                                                       tag="qkt", name=f"qkt{j}")
                        t = inflight[tkey]
                        nc.vector.tensor_copy(out=t[:, nh * 512:(nh + 1) * 512],
                                              in_=ps[:])
                        del inflight[key]
                        if (is_kt, nh) == (True, 1):
                            del inflight[("qkt", pnext, True)]
                            return None, t
                        if (is_kt, nh) == (False, 1):
                            qt = inflight.pop(("qkt", pnext, False))
                            return qt, None
                    return None, None

                def v_extra(j, mt):
                    """V matmuls for j6/j7 inside pair-0 slots (4 mm/slot),
                    psum borrowed from the (idle in pair 0) ppv ring."""
                    nh, step = (mt // 2) % 2, mt % 2
                    key = ("vx", j, nh)
                    if step == 0:
                        inflight[key] = ppv.tile([P, 512], F32, tag="ppv",
                                                 name=f"psvx{j}_{nh}")
                    ps = inflight[key]
                    for k in range(4 * step, 4 * step + 4):
                        nc.tensor.matmul(
                            ps[:],
                            lhsT=xs(k, j * P, P),
                            rhs=wv_sb[:, k * DIM + nh * 512:
                                      k * DIM + (nh + 1) * 512],
                            start=(k == 0), stop=(k == DP - 1),
                        )
                    if step == 1:
                        nc.vector.tensor_copy(
                            out=v_sb[j][:].rearrange(
                                "p (h x) -> p h x", x=VW)[:, 8 * nh:8 * nh + 8, :D_K],
                            in_=ps[:].rearrange("p (h d) -> p h d", d=D_K),
                        )
                        del inflight[key]

                for p in range(NPAIRS):
                    qt_next = kt_next = None
                    for mt in range(NP_T):
                        if p + 1 < NPAIRS:
                            qt, kt = filler(p + 1, mt)
                            if qt is not None:
                                qt_next = qt
                            if kt is not None:
                                kt_next = kt
                        if p == 0:
                            v_extra(6 + mt // 4, mt)
                        st_exp(p, mt)
                        if 0 < p < NPAIRS - 1:
                            pv_chunk(p - 1, mt)
                        elif p == NPAIRS - 1:
                            # front-load PV(6) into slots 0-3 so norm(6,h1)
                            # completes before the drain; dribble PV(7) h0/h1
                            # chunks through slots 4-7 as their exps land.
                            if mt < 4:
                                pv_chunk(p - 1, 2 * mt)
                                pv_chunk(p - 1, 2 * mt + 1)
                            elif mt == 4:
                                pv_chunk(p, 0, pool=fillp, mts=range(0, 4))
                            elif mt == 5:
                                pv_chunk(p, 2, mts=range(0, 4))
                                pv_chunk(p, 0, pool=fillp, mts=[4])
                            elif mt == 6:
                                pv_chunk(p, 0, pool=fillp, mts=[5])
                                pv_chunk(p, 2, mts=[4])
                                pv_chunk(p, 4, mts=range(0, 4))
                            else:
                                pv_chunk(p, 0, pool=fillp, mts=[6])
                                pv_chunk(p, 2, mts=[5])
                                pv_chunk(p, 4, mts=[4])
                                pv_chunk(p, 6, mts=range(0, 4))
                    qt_cur, kt_cur = qt_next, kt_next

                # ==== drain: close the dribbled PV(7) chunks and run the
                # projection interleaved; proj PSUMs alternate between the
                # pst ring and ppv halves so 3 j-tiles stay in flight while
                # waiting for attnT[7].
                last = NPAIRS - 1
                proj_ps = {}

                PROJ_SRC = {0: "pst", 1: "pst", 2: "ppv", 3: "mix",
                            4: "pst", 5: "pst", 6: "ppv", 7: "mix"}

                def proj_open(j):
                    kind = PROJ_SRC[j]
                    if kind == "ppv":
                        ps = [ppv.tile([P, 512], F32, tag="ppv",
                                       name=f"pso{j}_{nh}") for nh in range(2)]
                    elif kind == "mix":
                        ps = [fillp.tile([P, 512], F32, tag="fill",
                                         name=f"pso{j}_0"),
                              ppv.tile([P, 512], F32, tag="ppv",
                                       name=f"pso{j}_1")]
                    else:
                        t = pst.tile([P, DIM], F32, tag="pst", name=f"pso{j}")
                        ps = [t[:, 0:512], t[:, 512:1024]]
                    proj_ps[j] = ps
                    for p in range(NPAIRS - 1):
                        for nh in range(2):
                            nc.tensor.matmul(
                                ps[nh][:, :],
                                lhsT=attnT_sb[p][:, j * P:(j + 1) * P],
                                rhs=wout_sb[:, p * DIM + nh * 512:
                                            p * DIM + (nh + 1) * 512],
                                start=(p == 0), stop=False,
                            )

                def proj_close(j):
                    ps = proj_ps.pop(j)
                    for nh in range(2):
                        nc.tensor.matmul(
                            ps[nh][:, :],
                            lhsT=attnT_sb[last][:, j * P:(j + 1) * P],
                            rhs=wout_sb[:, last * DIM + nh * 512:
                                        last * DIM + (nh + 1) * 512],
                            start=False, stop=True,
                        )
                    o = evp.tile([P, DIM], F32, tag="out", name=f"o{j}")
                    for nh in range(2):
                        nc.vector.tensor_add(
                            out=o[:, nh * 512:(nh + 1) * 512], in0=ps[nh][:],
                            in1=bias_bc[:, nh * 512:(nh + 1) * 512])
                    nc.sync.dma_start(out[j * P:(j + 1) * P, :], o[:])

                proj_open(0)
                pv_chunk(last, 0, pool=fillp, mts=[7])
                pv_chunk(last, 2, mts=[6, 7])
                pv_chunk(last, 4, mts=[5, 6, 7])
                pv_chunk(last, 6, mts=[4, 5, 6, 7])
                proj_open(1)
                proj_open(2)
                proj_open(3)
                proj_close(0)
                proj_open(4)
                proj_close(1)
                proj_open(5)
                proj_close(2)
                proj_open(6)
                proj_close(3)
                proj_open(7)
                proj_close(4)
                proj_close(5)
                proj_close(6)
                proj_close(7)

    nc.compile()
    return nc


_NC_CACHE = None


def _get_program():
    global _NC_CACHE
    if _NC_CACHE is None:
        _NC_CACHE = build_program()
    return _NC_CACHE


def make_in_maps(x, w_qkv, w_out, b_out):
    w_qkv_c = np.ascontiguousarray(w_qkv).astype(_NPDT)
    w_out_c = np.ascontiguousarray(w_out).astype(_NPDT)
    b_out_c = np.ascontiguousarray(b_out).astype(np.float32)
    in_maps = []
    for b in range(N_CORES):
        xTb = np.ascontiguousarray(np.asarray(x[b]).T).astype(_NPDT)
        in_maps.append({
            "xT": xTb,
            "w_qkv": w_qkv_c,
            "w_out": w_out_c,
            "b_out": b_out_c,
        })
    return in_maps


def kernel(x, w_qkv, w_out, b_out):
    nc = _get_program()
    in_maps = make_in_maps(x, w_qkv, w_out, b_out)
    res = run_bass_kernel_spmd(nc, in_maps, list(range(N_CORES)))
    outs = [np.asarray(r["out"], dtype=np.float32) for r in res.results]
    return np.stack(outs, axis=0)
